# revision 35
# baseline (speedup 1.0000x reference)
"""Trainium2 Bass kernel for nn_Attn: additive-attention scores + softmax.

Reference computation (S=512, B=64, H=1024):
    e = relu(concat([hidden bcast, enc], -1) @ Wa^T + ba)      # (S,B,H)
    score = (log(S)/sqrt(H)) * (e @ Ws^T)[...,0]               # (S,B)
    attn = softmax(score.T + pe  with seq_mask -> -1e12, axis=S)  # (B,1,S)

Strategy: data-parallel over B across 8 cores (8 batches each); the concat
splits algebraically into enc @ Wa2^T + c[b] with c = hidden @ Wa1^T + ba
(tiny, fp16). The big matmul runs in e^T orientation with s on PSUM
partitions and h on the free axis, in fp8e4m3 MatmulPerfMode.DoubleRow
(k-pairs, 2x rate; the PE stream stays mode-uniform — mixing DR with fp32r
back-to-back corrupts on hw).

The Ws score reduction costs ZERO matmuls: |Ws_h| (and a global scale G) is
folded into the Wa2/Wa1/ba columns on the host, columns are permuted so
positive-Ws columns come first (split point P is a build parameter computed
from Ws at kernel() time), and the relu ACT's accum_out sums each sign range
along the free axis. score = (pos-sum) - (neg-sum), computed per
(batch, s-block) into per-partition columns, then transposed back to (b, s)
rows via a 16KB DRAM round-trip DMA (DMA-side transposes avoid is_transpose
PE mode mixing).

The per-batch bias c lands on the free (h) axis, where the ACT bias cannot
reach: c is computed as today ([128, 8] tiles), DMA'd to DRAM transposed,
and read back with a stride-0 partition-broadcast DMA ([1, H] row -> all 128
partitions; verified bit-exact on hw), then added to the z PSUM by the DVE.

Scaling: wa2/wa1/ba carry G*|Ws_h| (G=512 clears e4m3/fp16 subnormals), so
accumulated scores are G*score; pe is host-scaled by G/SCALE and the 1/G
with the softmax temperature folds into the epilogue exp scale. The seq
mask is folded into ped (-1e12). Measured rel err ~1e-2 vs the 2e-2 gate.
"""
import math
import sys

sys.path.insert(0, "/opt/trn_rl_repo")

import numpy as np
import ml_dtypes

import concourse.bacc as bacc
import concourse.bass as bass
import concourse.mybir as mybir
import concourse.tile as tile
from concourse.bass_utils import run_bass_kernel_spmd

S, B, H = 512, 64, 1024
NCORES = 8
BLOC = B // NCORES          # 8 batches per core
KT = H // 128               # 8 contraction tiles
HT = H // 128               # 8 h tiles
SBK = S // 128              # 4 s-blocks per batch
SCALE = math.log(S) / math.sqrt(H)
GSCALE = 512.0              # global scale folded with |Ws| into Wa2/Wa1/ba

F32R = mybir.dt.float32r
F16 = mybir.dt.float16
F8 = mybir.dt.float8e4
F32 = mybir.dt.float32
U8 = mybir.dt.uint8
AF = mybir.ActivationFunctionType
DR = mybir.MatmulPerfMode.DoubleRow

_SPLIT = None  # #positive-Ws columns; set by make_in_maps, used by build_nc


def build_nc(reps=1, split=None):
    """reps>1 wraps the whole body in a hardware loop — used only for timing."""
    if split is None:
        split = _SPLIT
    assert split is not None, "call make_in_maps first (computes the Ws sign split)"
    nc = bacc.Bacc("TRN2", target_bir_lowering=False, debug=False,
                   num_devices=NCORES)
    # enc^T, fp8: [b, k, p, s]
    xt = nc.dram_tensor("xt", [BLOC, KT, 128, S], F8, kind="ExternalInput").ap()
    # G*|Ws|-folded, sign-permuted Wa2^T fp8: [k, p, h']
    wa2t = nc.dram_tensor("wa2t", [KT, 128, H], F8, kind="ExternalInput").ap()
    # G*|Ws|-folded, sign-permuted Wa1^T fp16: [k, p, h']
    wa1t = nc.dram_tensor("wa1t", [KT, 128, H], F16, kind="ExternalInput").ap()
    ht = nc.dram_tensor("ht", [H, BLOC], F16, kind="ExternalInput").ap()
    ba = nc.dram_tensor("ba", [H, 1], F32, kind="ExternalInput").ap()
    # pe*G/SCALE with mask folded in as -1e12
    ped = nc.dram_tensor("ped", [BLOC, S], F32, kind="ExternalInput").ap()
    outp = nc.dram_tensor("out", [BLOC, S], F32, kind="ExternalOutput").ap()
    # scratch: c rows for the broadcast read-back; score transpose bounce
    c2d = nc.dram_tensor("c2d", [BLOC, H], F32, kind="Internal").ap()
    std = nc.dram_tensor("std", [128, SBK * BLOC], F32, kind="Internal").ap()

    with tile.TileContext(nc) as tc:
        with tc.tile_pool(name="wpool", bufs=1) as wpool, \
             tc.tile_pool(name="xpool", bufs=3) as xpool, \
             tc.tile_pool(name="epool", bufs=3) as epool, \
             tc.tile_pool(name="spool", bufs=1) as spool, \
             tc.tile_pool(name="eps", bufs=4, space="PSUM") as eps, \
             tc.tile_pool(name="cps", bufs=2, space="PSUM") as cps:

          def emit_body():
            # ---- DMAs: wa1/ht first (cT is first on PE), then x/wa2 ----
            ht_sb = []
            for k in range(KT):
                t = wpool.tile([128, BLOC], F16, tag=f"ht_{k}")
                nc.sync.dma_start(t[:], ht[k * 128:(k + 1) * 128, :])
                ht_sb.append(t)
            wa1_sb = []
            for k in range(KT):
                w1 = wpool.tile([128, H], F16, tag=f"wa1_{k}")
                nc.sync.dma_start(w1[:], wa1t[k])
                wa1_sb.append(w1)
            ba_sb = wpool.tile([128, HT], F32, tag="ba")
            nc.sync.dma_start(ba_sb[:], ba.rearrange("(k p) o -> p (k o)", p=128))

            wa2_sb = wpool.tile([128, KT, H], F8, tag="wa2", bufs=2)
            x_sb = xpool.tile([128, KT, S], F8, tag="x")
            for kk in range(0, KT, 2):
                nc.sync.dma_start(
                    x_sb[:, kk:kk + 2, :],
                    xt[0, kk:kk + 2].rearrange("k p s -> p k s"))
                nc.sync.dma_start(
                    wa2_sb[:, kk:kk + 2, :],
                    wa2t[kk:kk + 2].rearrange("k p h -> p k h"))
            ped_sb = spool.tile([BLOC, S], F32, tag="ped")
            nc.sync.dma_start(ped_sb[:], ped)

            # ---- c' = G*|Ws|*(Wa1 @ hidden^T + ba): [128, 8] tiles, then
            # bounce through DRAM into per-batch broadcast rows ----
            for h in range(HT):
                cp = cps.tile([128, BLOC], F32, tag="cps")
                for k in range(KT):
                    nc.tensor.matmul(cp[:], wa1_sb[k][:, h * 128:(h + 1) * 128],
                                     ht_sb[k][:],
                                     start=(k == 0), stop=(k == KT - 1))
                ct = wpool.tile([128, BLOC], F32, tag=f"ct_{h}")
                nc.vector.tensor_scalar_add(ct[:], cp[:], ba_sb[:, h:h + 1])
                nc.sync.dma_start(
                    c2d[:, h * 128:(h + 1) * 128].rearrange("b p -> p b"), ct[:])
            cb_sb = []
            for b in range(BLOC):
                cb = wpool.tile([128, H], F32, tag=f"cb_{b}")
                row = c2d[b]
                bsrc = bass.AP(row.tensor, row.offset, [[0, 128]] + list(row.ap))
                nc.sync.dma_start(cb[:], bsrc)
                cb_sb.append(cb)

            # accum collectors: 4 col-groups (pos/neg x lo/hi half), 32 cols
            # each (col = sblk*8 + b).  Zeroed every rep: a sign group can be
            # empty for one half, and accum_out must not carry across reps.
            sacc = spool.tile([128, 4 * SBK * BLOC], F32, tag="sacc")
            nc.vector.memset(sacc[:], 0.0)
            NC_ = SBK * BLOC
            # per 512-col half: [pos span, neg span] in local coords
            spans = [(min(split, 512), 512), (max(split - 512, 0), 512)]

            # ---- main loop: z^T tiles [128 s, 512 h-half] ----
            for b in range(BLOC):
                if b > 0:
                    x_sb = xpool.tile([128, KT, S], F8, tag="x")
                    for kk in range(0, KT, 2):
                        nc.sync.dma_start(
                            x_sb[:, kk:kk + 2, :],
                            xt[b, kk:kk + 2].rearrange("k p s -> p k s"))
                for sblk in range(SBK):
                    col = sblk * BLOC + b
                    for half in range(2):
                        zp = eps.tile([128, 512], F32, tag="zp")
                        for kk in range(0, KT, 2):
                            nc.tensor.matmul(
                                zp[:],
                                x_sb[:, kk:kk + 2, sblk * 128:(sblk + 1) * 128],
                                wa2_sb[:, kk:kk + 2, half * 512:(half + 1) * 512],
                                start=(kk == 0), stop=(kk == KT - 2),
                                perf_mode=DR)
                        nc.vector.tensor_tensor(
                            out=zp[:], in0=zp[:],
                            in1=cb_sb[b][:, half * 512:(half + 1) * 512],
                            op=mybir.AluOpType.add)
                        ps, ns = spans[half]
                        scr = epool.tile([128, 512], F8, tag="scr")
                        if ps > 0:
                            nc.scalar.activation(
                                scr[:, 0:ps], zp[:, 0:ps], AF.Relu,
                                accum_out=sacc[:, half * NC_ + col:
                                               half * NC_ + col + 1])
                        if ns > ps:
                            nc.scalar.activation(
                                scr[:, ps:ns], zp[:, ps:ns], AF.Relu,
                                accum_out=sacc[:, (2 + half) * NC_ + col:
                                               (2 + half) * NC_ + col + 1])

            # score*G = (pos_lo + pos_hi) - (neg_lo + neg_hi)  -> [128, 32]
            stot = spool.tile([128, NC_], F32, tag="stot")
            nc.vector.tensor_tensor(out=stot[:], in0=sacc[:, 0:NC_],
                                    in1=sacc[:, NC_:2 * NC_],
                                    op=mybir.AluOpType.add)
            nc.vector.tensor_tensor(out=stot[:], in0=stot[:],
                                    in1=sacc[:, 2 * NC_:3 * NC_],
                                    op=mybir.AluOpType.subtract)
            nc.vector.tensor_tensor(out=stot[:], in0=stot[:],
                                    in1=sacc[:, 3 * NC_:4 * NC_],
                                    op=mybir.AluOpType.subtract)
            # transpose+regroup [128 s, (sblk,b)] -> [b, sblk*128+s] entirely
            # inside the DRAM bounce (DVE lanes cannot cross partitions)
            nc.sync.dma_start(std, stot[:])
            t_pre = spool.tile([BLOC, SBK, 128], F32, tag="t_pre")
            for sblk in range(SBK):
                nc.sync.dma_start(
                    t_pre[:, sblk, :],
                    std[:, sblk * BLOC:(sblk + 1) * BLOC].rearrange("p b -> b p"))

            # ---- epilogue: t = G*score + ped ; softmax((SCALE/G)*t) ----
            t_sb = spool.tile([BLOC, S], F32, tag="t")
            nc.vector.tensor_tensor(
                out=t_sb[:], in0=t_pre.rearrange("b s p -> b (s p)"),
                in1=ped_sb[:], op=mybir.AluOpType.add)
            u_sb = spool.tile([BLOC, S], F32, tag="u")
            esum = spool.tile([BLOC, 1], F32, tag="esum")
            nc.scalar.activation(u_sb[:], t_sb[:], AF.Exp,
                                 scale=SCALE / GSCALE, accum_out=esum[:])
            rcp = spool.tile([BLOC, 1], F32, tag="rcp")
            nc.vector.reciprocal(rcp[:], esum[:])
            o_sb = spool.tile([BLOC, S], F32, tag="o")
            nc.vector.tensor_scalar_mul(o_sb[:], u_sb[:], rcp[:])
            nc.sync.dma_start(outp, o_sb[:])

          if reps == 1:
              emit_body()
          else:
              from concourse.engine_type import EngineType
              with tc.For_i(0, reps, 1, hint_engines=(EngineType.PE,)):
                  emit_body()

    nc.compile()
    return nc


def make_in_maps(hidden, encoder_outputs, pe, seq_mask, Wa, ba, Ws):
    """Host-side sharding + layout prep: transposes/casts, the |Ws|/G fold
    (exact rescaling undone in the epilogue exp scale), and the Ws-sign
    column permutation."""
    global _SPLIT
    hidden = np.asarray(hidden, dtype=np.float32)
    enc = np.asarray(encoder_outputs, dtype=np.float32)
    pe = np.asarray(pe, dtype=np.float32)
    seq_mask = np.asarray(seq_mask)
    Wa = np.asarray(Wa, dtype=np.float32)
    ba = np.asarray(ba, dtype=np.float32)
    Ws = np.asarray(Ws, dtype=np.float32)[0]
    F8NP = ml_dtypes.float8_e4m3

    perm = np.argsort(Ws < 0, kind="stable")   # positive/zero first
    _SPLIT = int((Ws >= 0).sum())
    fold = (np.float32(GSCALE) * np.abs(Ws[perm])).astype(np.float32)  # [H]

    # (H_out, H_in) -> [k, p, h'] = W^T, permuted+folded along h'
    wa1t = np.ascontiguousarray(
        (Wa[perm, :H] * fold[:, None]).T.reshape(KT, 128, H)).astype(np.float16)
    wa2t = np.ascontiguousarray(
        (Wa[perm, H:] * fold[:, None]).T.reshape(KT, 128, H)).astype(F8NP)
    ba_col = np.ascontiguousarray((ba[perm] * fold).reshape(H, 1))
    ped_all = np.where(seq_mask, np.float32(-1e12),
                       pe * np.float32(GSCALE / SCALE)).astype(np.float32)

    in_maps = []
    for c in range(NCORES):
        bsl = slice(c * BLOC, (c + 1) * BLOC)
        xt = np.ascontiguousarray(
            enc[:, bsl, :].transpose(1, 2, 0)).reshape(BLOC, KT, 128, S).astype(F8NP)
        htc = np.ascontiguousarray(hidden[0, bsl, :].T).astype(np.float16)
        in_maps.append({
            "xt": xt, "wa2t": wa2t, "wa1t": wa1t, "ht": htc,
            "ba": ba_col, "ped": np.ascontiguousarray(ped_all[bsl]),
        })
    return in_maps


_NC_CACHE = None


def kernel(hidden, encoder_outputs, pe, seq_mask, Wa, ba, Ws):
    global _NC_CACHE
    in_maps = make_in_maps(hidden, encoder_outputs, pe, seq_mask, Wa, ba, Ws)
    if _NC_CACHE is None:
        _NC_CACHE = build_nc()
    nc = _NC_CACHE
    res = run_bass_kernel_spmd(nc, in_maps, list(range(NCORES)))
    attn = np.concatenate([res.results[c]["out"] for c in range(NCORES)], axis=0)
    return attn[:, None, :].astype(np.float32)
